# revision 7
# baseline (speedup 1.0000x reference)
"""Trainium2 Bass kernel for the GRU decoder problem.

Math (reference):
    emb[b,t]   = W_emb @ message[b,t] + b_emb                  # [B,T,E]
    xs[t]      = init_emb (t=0) else emb[:, t-1]               # GRU inputs
    gi[t]      = W_ih @ xs[t] + b_ih                           # [B,3H]
    gh         = W_hh @ h + b_hh
    r          = sigmoid(gi_r + gh_r); z = sigmoid(gi_z + gh_z)
    n          = tanh(gi_n + r * gh_n)
    h'         = (1-z)*n + z*h
    out        = sigmoid(W_fc2 @ elu(W_fc1 @ h_T + b_fc1) + b_fc2)

Device strategy (pure data parallel over batch, 8 cores, B/core = 512):
  - message host-transposed to [t, p, c4, b] fp16 per core so each DMA
    reads 4KB contiguous per partition; the final token's embedding is
    never consumed, so only t = 0..62 ships.
  - Step 0 is batch-independent (h0 = 0, x0 = init_emb): h1 is computed
    on the host and broadcast; the device scan runs steps 1..63.
  - Recurrent state is m := 1 + h (fp16).  Folds (all host-side):
      * z-gate weights negated        -> sigmoid gives zc = 1-z directly,
        so one ACT op evaluates [r | zc] from one PSUM bank.
      * n-gate weights doubled        -> sigma(2s) = (tanh(s)+1)/2 = n^,
        so tanh becomes a second sigmoid (single ACT table set).
      * all gate biases + W_hh row-sums (from the m-shift) folded into a
        33rd contraction row of the input-side weights (emb tiles carry a
        persistent ones-row); the n-gate h-side bias rides the free scalar
        slot of the rh STT.
    Update: m' = 2*(zc .* n^) - (zc-1) .* m   [2 DVE STT/TT + 1 GPSIMD STT]
  - Engine split per step/chain: PE 6 gate MMs + 4 emb MMs; ACT the two
    sigmoids; DVE rh/s2/u2/m' + half the emb eviction; GPSIMD negg.
  - Two 256-wide batch chains hide the serial per-step latency.
"""

import numpy as np

import concourse.bass as bass
import concourse.tile as tile
import concourse.mybir as mybir
from concourse.bass_utils import run_bass_kernel_spmd

N_CORES = 8
B, T, V, E, H, FC, O = 4096, 64, 512, 32, 128, 256, 1024
BS = B // N_CORES      # batch per core
TS = T - 1             # message slices consumed by the GRU
NCH = 2                # batch chains per core
CW = BS // NCH         # chain width
LOOK = 6               # emb pipeline lookahead (steps)
EMB_D = 8              # emb_aug ring depth

F16 = mybir.dt.float16
F32 = mybir.dt.float32
AF = mybir.ActivationFunctionType
OP = mybir.AluOpType

_PROGRAM = None
LAST_RESULTS = None


# walrus codegen encodes at most 1 sem wait per instruction; excess waits
# are hoisted onto NoOp carriers on the same engine (engines execute their
# queues in order, so waiting earlier on the same engine is equivalent).
_WAIT_LIMITS: dict = {}
_DEFAULT_WAIT_LIMIT = 1


def _split_excess_waits(nc):
    for f in nc.m.functions:
        for bb in f.blocks:
            newlist = []
            changed = False
            for inst in bb.instructions:
                si = inst.sync_info
                limit = _WAIT_LIMITS.get(type(inst).__name__, _DEFAULT_WAIT_LIMIT)
                if (
                    limit is not None
                    and si is not None
                    and si.on_wait is not None
                    and len(si.on_wait) > limit
                ):
                    waits = list(si.on_wait)
                    for k, w in enumerate(waits[:-limit]):
                        carrier = mybir.InstNoOp(
                            name=f"{inst.name}-wsplit{k}", ins=[], outs=[]
                        )
                        carrier.engine = inst.engine
                        carrier.sync_info = mybir.SyncInfo(on_wait=[w], on_update=[])
                        newlist.append(carrier)
                    si.on_wait = waits[-limit:]
                    inst.sync_info = si
                    changed = True
                newlist.append(inst)
            if changed:
                bb.instructions[:] = newlist


def _build_program():
    nc = bass.Bass()

    msg = nc.dram_tensor("msg", [TS, 128, V // 128, BS], F16, kind="ExternalInput")
    wembT = nc.dram_tensor("wembT", [V, E], F16, kind="ExternalInput")
    wihT = nc.dram_tensor("wihT", [E + 1, 3 * H], F16, kind="ExternalInput")
    whhT = nc.dram_tensor("whhT", [H, 3 * H], F16, kind="ExternalInput")
    wfc1T = nc.dram_tensor("wfc1T", [H, FC], F16, kind="ExternalInput")
    wfc2T = nc.dram_tensor("wfc2T", [FC, O], F16, kind="ExternalInput")
    # bias columns: 0 m1, 1 rh-stt, 2..3 fc1, 4..11 fc2
    biasd = nc.dram_tensor("bias", [128, 12], F32, kind="ExternalInput")
    out = nc.dram_tensor("out", [O // 128, 128, BS], F32, kind="ExternalOutput")

    with tile.TileContext(nc) as tc:
        with (
            tc.tile_pool(name="const", bufs=1) as const,
            tc.tile_pool(name="msgp", bufs=8) as msgp,
            tc.tile_pool(name="rzs", bufs=3) as rzsp,
            tc.tile_pool(name="gate", bufs=3) as gate,
            tc.tile_pool(name="fcp", bufs=2) as fcp,
            tc.tile_pool(name="outp", bufs=2) as outp,
            tc.tile_pool(name="ps_rza", bufs=2, space="PSUM") as ps_rza,
            tc.tile_pool(name="ps_rzb", bufs=2, space="PSUM") as ps_rzb,
            tc.tile_pool(name="ps_pna", bufs=1, space="PSUM") as ps_pna,
            tc.tile_pool(name="ps_pnb", bufs=1, space="PSUM") as ps_pnb,
            tc.tile_pool(name="ps_emb", bufs=2, space="PSUM") as ps_emb,
        ):
            ps_rz = [ps_rza, ps_rzb]
            ps_pn = [ps_pna, ps_pnb]

            # ---- resident constants ----
            wemb_sb = const.tile([128, V // 128, E], F16)
            nc.sync.dma_start(wemb_sb[:], wembT.rearrange("(c p) e -> p c e", p=128))
            wih_sb = const.tile([E + 1, 3 * H], F16)
            nc.sync.dma_start(wih_sb[:], wihT[:])
            whh_sb = const.tile([H, 3 * H], F16)
            nc.sync.dma_start(whh_sb[:], whhT[:])
            wfc1_sb = const.tile([H, FC], F16)
            nc.sync.dma_start(wfc1_sb[:], wfc1T[:])
            wfc2_sb = const.tile([128, FC // 128, O], F16)
            nc.sync.dma_start(wfc2_sb[:], wfc2T.rearrange("(c p) o -> p c o", p=128))
            bias_sb = const.tile([128, 12], F32)
            nc.sync.dma_start(bias_sb[:], biasd[:])
            zeros = const.tile([128, BS], F16)
            nc.gpsimd.memset(zeros[:], 0.0)

            # emb_aug ring: rows 0..31 emb (rewritten per slot use), row 32
            # a persistent 1.0 (bias contraction row), written once here.
            emb_ring = []
            for i in range(EMB_D):
                tl = const.tile([E + 1, BS], F16, name=f"embr{i}", tag=f"embr{i}")
                nc.gpsimd.memset(tl[E : E + 1, :], 1.0)
                emb_ring.append(tl)

            # m after step 0 is batch-independent (host-computed) -> broadcast.
            m_tiles = [[None, None] for _ in range(NCH)]
            for c in range(NCH):
                m0 = const.tile([H, CW], F16, name=f"m0_{c}")
                nc.vector.tensor_scalar_add(m0[:], zeros[:, 0:CW], bias_sb[:, 0:1])
                m_tiles[c][0] = m0
                m_tiles[c][1] = const.tile([H, CW], F16, name=f"m1_{c}")

            def emb_front(k):
                """DMA message slice k and project it to emb_ring[k % EMB_D]."""
                mt = msgp.tile([128, V // 128, BS], F16)
                nc.sync.dma_start(mt[:], msg[k])
                ep = ps_emb.tile([E, BS], F32)
                for c4 in range(V // 128):
                    nc.tensor.matmul(
                        ep[:],
                        wemb_sb[:, c4, :],
                        mt[:, c4, :],
                        start=(c4 == 0),
                        stop=(c4 == V // 128 - 1),
                    )
                es = emb_ring[k % EMB_D]
                # eviction split across DVE and ACT to balance load
                nc.vector.tensor_copy(es[0:E, 0:CW], ep[:, 0:CW])
                nc.scalar.copy(es[0:E, CW:BS], ep[:, CW:BS])

            for k in range(T + LOOK):
                if k < TS:
                    emb_front(k)

                st = k - LOOK
                if not (1 <= st <= T - 1):
                    continue
                es = emb_ring[(st - 1) % EMB_D]
                for c in range(NCH):
                    sl = bass.ts(c, CW)
                    m_prev = m_tiles[c][(st - 1) % 2]
                    m_new = m_tiles[c][st % 2]

                    rz = ps_rz[c].tile([128, 2 * CW], F32, tag="rz")
                    pn = ps_pn[c].tile([128, 2 * CW], F32, tag="pn")
                    # NOTE: start=True clears has_written for the WHOLE bank,
                    # so accumulation groups within a bank must be serialized:
                    # close the r group (gi+gh) before opening the z group.
                    nc.tensor.matmul(pn[:, 0:CW], wih_sb[:, 2 * H:3 * H],
                                     es[:, sl], start=True, stop=True)
                    nc.tensor.matmul(rz[:, 0:CW], wih_sb[:, 0:H], es[:, sl],
                                     start=True, stop=False)
                    nc.tensor.matmul(rz[:, 0:CW], whh_sb[:, 0:H], m_prev[:],
                                     start=False, stop=True)
                    nc.tensor.matmul(rz[:, CW:2 * CW], wih_sb[:, H:2 * H],
                                     es[:, sl], start=True, stop=False)
                    nc.tensor.matmul(rz[:, CW:2 * CW], whh_sb[:, H:2 * H],
                                     m_prev[:], start=False, stop=True)
                    nc.tensor.matmul(pn[:, CW:2 * CW], whh_sb[:, 2 * H:3 * H],
                                     m_prev[:], start=True, stop=True)

                    # [r | zc] in one sigmoid (z-weights pre-negated)
                    rzs = rzsp.tile([128, 2 * CW], F16, tag="rzs" + str(c))
                    nc.scalar.activation(rzs[:], rz[:], AF.Sigmoid)
                    # g = (1 - zc) .* m   (GPSIMD, off the serial path)
                    v = gate.tile([128, CW], F16, tag="v" + str(c))
                    nc.gpsimd.tensor_tensor(v[:], rzs[:, CW:2 * CW], m_prev[:],
                                            op=OP.mult)
                    g = gate.tile([128, CW], F16, tag="g" + str(c))
                    nc.gpsimd.tensor_tensor(g[:], m_prev[:], v[:],
                                            op=OP.subtract)
                    # rh = (2*ghn' + c_hn) .* r
                    rh = gate.tile([128, CW], F16, tag="rh" + str(c))
                    nc.vector.scalar_tensor_tensor(
                        rh[:], pn[:, CW:2 * CW], bias_sb[:, 1:2], rzs[:, 0:CW],
                        op0=OP.add, op1=OP.mult)
                    # s2 = 2*gin' + rh  (= 2s)
                    s2 = gate.tile([128, CW], F16, tag="s2" + str(c))
                    nc.vector.tensor_tensor(s2[:], pn[:, 0:CW], rh[:], op=OP.add)
                    # n^ = sigmoid(2s)
                    nh = gate.tile([128, CW], F16, tag="nh" + str(c))
                    nc.scalar.activation(nh[:], s2[:], AF.Sigmoid)
                    # u2 = n^ .* zc ;  m' = 2*u2 + g
                    u2 = gate.tile([128, CW], F16, tag="u2" + str(c))
                    nc.vector.tensor_tensor(u2[:], nh[:], rzs[:, CW:2 * CW],
                                            op=OP.mult)
                    nc.vector.scalar_tensor_tensor(
                        m_new[:], u2[:], 2.0, g[:],
                        op0=OP.mult, op1=OP.add)

            # ---- output head: fc1 + ELU, fc2 + sigmoid ----
            m_last = [m_tiles[c][(T - 1) % 2] for c in range(NCH)]
            hid = []
            for f in range(FC // 128):
                pf = ps_rz[0].tile([128, BS], F32, tag="rz")
                for c in range(NCH):
                    nc.tensor.matmul(pf[:, bass.ts(c, CW)],
                                     wfc1_sb[:, bass.ts(f, 128)],
                                     m_last[c][:], start=True, stop=True)
                bcol = bias_sb[:, 2 + f:3 + f]
                x1 = fcp.tile([128, BS], F16, tag="fcx")
                nc.vector.tensor_scalar_add(x1[:], pf[:], bcol)
                e1 = fcp.tile([128, BS], F32, tag="fce")
                nc.scalar.activation(e1[:], pf[:], AF.Exp, bias=bcol)
                # elu(x) = max(x,0) + min(exp(x)-1, 0)
                em = fcp.tile([128, BS], F16, tag="fcm")
                nc.vector.scalar_tensor_tensor(em[:], e1[:], -1.0, zeros[:],
                                               op0=OP.add, op1=OP.min)
                hc = fcp.tile([128, BS], F16, tag="hid" + str(f))
                nc.vector.scalar_tensor_tensor(hc[:], x1[:], 0.0, em[:],
                                               op0=OP.max, op1=OP.add)
                hid.append(hc)
            for o in range(O // 128):
                po = ps_rz[1].tile([128, BS], F32, tag="rz")
                for f in range(FC // 128):
                    nc.tensor.matmul(po[:], wfc2_sb[:, f, bass.ts(o, 128)],
                                     hid[f][:], start=(f == 0),
                                     stop=(f == FC // 128 - 1))
                ob = outp.tile([128, BS], F32)
                nc.scalar.activation(ob[:], po[:], AF.Sigmoid,
                                     bias=bias_sb[:, 4 + o:5 + o])
                nc.sync.dma_start(out[o], ob[:])

    _split_excess_waits(nc)
    return nc


def _sigmoid(x):
    return 1.0 / (1.0 + np.exp(-x))


def kernel(message, W_emb, b_emb, init_emb, W_ih, W_hh, b_ih, b_hh,
           W_fc1, b_fc1, W_fc2, b_fc2, _trace=False, _trace_kwargs=None):
    global _PROGRAM, LAST_RESULTS
    if _PROGRAM is None:
        _PROGRAM = _build_program()
    nc = _PROGRAM

    f32 = np.float32
    f16 = np.float16
    f64 = np.float64

    # message -> per-core [t, p, c4, b] fp16 (v = c4*128 + p); the final
    # token's embedding is never consumed by the GRU.
    m16 = message.astype(f16)
    msgT = np.ascontiguousarray(
        m16[:, :TS, :]
        .reshape(N_CORES, BS, TS, V // 128, 128)
        .transpose(0, 2, 4, 3, 1)
    )

    Wih_r, Wih_z, Wih_n = (W_ih[:H].astype(f64), W_ih[H:2 * H].astype(f64),
                           W_ih[2 * H:].astype(f64))
    Whh_r, Whh_z, Whh_n = (W_hh[:H].astype(f64), W_hh[H:2 * H].astype(f64),
                           W_hh[2 * H:].astype(f64))
    b_emb64 = b_emb.astype(f64)
    b_r = b_ih[:H].astype(f64) + b_hh[:H] + Wih_r @ b_emb64
    b_z = b_ih[H:2 * H].astype(f64) + b_hh[H:2 * H] + Wih_z @ b_emb64
    b_in = b_ih[2 * H:].astype(f64) + Wih_n @ b_emb64
    b_hn = b_hh[2 * H:].astype(f64)

    # m-state folds (h = m - 1): subtract W_hh row sums from the biases
    bias_row = np.concatenate([
        b_r - Whh_r.sum(1),
        -(b_z - Whh_z.sum(1)),
        2.0 * b_in,
    ])
    wih_eff = np.concatenate([Wih_r, -Wih_z, 2.0 * Wih_n], 0)      # [3H, E]
    whh_eff = np.concatenate([Whh_r, -Whh_z, 2.0 * Whh_n], 0)      # [3H, H]
    wihT = np.ascontiguousarray(
        np.concatenate([wih_eff, bias_row[:, None]], 1).T).astype(f16)
    whhT = np.ascontiguousarray(whh_eff.T).astype(f16)
    wembT = np.ascontiguousarray(W_emb.T).astype(f16)
    wfc1T = np.ascontiguousarray(W_fc1.T).astype(f16)
    wfc2T = np.ascontiguousarray(W_fc2.T).astype(f16)

    # step 0 is batch-independent: h0 = 0, x0 = init_emb
    gi0 = W_ih.astype(f64) @ init_emb[0, 0].astype(f64) + b_ih
    r0 = _sigmoid(gi0[0:H] + b_hh[0:H])
    z0 = _sigmoid(gi0[H:2 * H] + b_hh[H:2 * H])
    n0 = np.tanh(gi0[2 * H:3 * H] + r0 * b_hh[2 * H:3 * H])
    h1 = (1.0 - z0) * n0

    bias = np.zeros((128, 12), f32)
    bias[:, 0] = 1.0 + h1
    bias[:, 1] = 2.0 * (b_hn - Whh_n.sum(1))
    bias[:, 2:4] = (b_fc1.astype(f64) - W_fc1.astype(f64).sum(1)) \
        .reshape(2, 128).T.astype(f32)
    bias[:, 4:12] = b_fc2.reshape(8, 128).T.astype(f32)

    shared = dict(wembT=wembT, wihT=wihT, whhT=whhT, wfc1T=wfc1T,
                  wfc2T=wfc2T, bias=bias)
    in_maps = [dict(msg=msgT[c], **shared) for c in range(N_CORES)]

    kw = dict(_trace_kwargs or {})
    res = run_bass_kernel_spmd(nc, in_maps, list(range(N_CORES)),
                               trace=_trace, **kw)
    LAST_RESULTS = res

    outs = [res.results[c]["out"].reshape(O, BS).T for c in range(N_CORES)]
    return np.ascontiguousarray(np.concatenate(outs, axis=0), dtype=f32)


# revision 8
# speedup vs baseline: 1.2567x; 1.2567x over previous
"""Trainium2 Bass kernel for the GRU decoder problem.

Math (reference):
    emb[b,t]   = W_emb @ message[b,t] + b_emb                  # [B,T,E]
    xs[t]      = init_emb (t=0) else emb[:, t-1]               # GRU inputs
    gi[t]      = W_ih @ xs[t] + b_ih                           # [B,3H]
    gh         = W_hh @ h + b_hh
    r          = sigmoid(gi_r + gh_r); z = sigmoid(gi_z + gh_z)
    n          = tanh(gi_n + r * gh_n)
    h'         = (1-z)*n + z*h
    out        = sigmoid(W_fc2 @ elu(W_fc1 @ h_T + b_fc1) + b_fc2)

Device strategy (pure data parallel over batch, 8 cores, B/core = 512):
  - message host-transposed to [t, p, c4, b] fp16 per core so each DMA
    reads 4KB contiguous per partition; the final token's embedding is
    never consumed, so only t = 0..62 ships.
  - Step 0 is batch-independent (h0 = 0, x0 = init_emb): h1 is computed
    on the host and broadcast; the device scan runs steps 1..63.
  - Recurrent state is mh := (1 + h)/2 in fp16.  Folds (all host-side):
      * z-gate weights negated        -> sigmoid gives zc = 1-z directly,
        so one ACT op evaluates [r | zc].
      * n-gate weights doubled        -> sigma(2s) = (tanh(s)+1)/2 = n^,
        so tanh becomes a second sigmoid (single ACT table set).
      * h = 2*mh - 1 absorbed into doubled W_hh + row-sum bias folds; all
        gate biases ride a 33rd contraction row of the input-side weights
        (emb tiles carry a persistent ones-row) or the rh STT scalar.
    Update is pure tensor_tensor:  d = n^ - mh ; e = zc.*d ; mh' = mh + e.
  - PSUM layout: two double-bank tiles per step so each bank holds at
    most one accumulation chain (start=True clears has_written bank-wide):
      big1 = [r_A | r_B | zc_A | zc_B]   (gi_r/gi_z wide, gh_* accumulate)
      big2 = [gin_A | gin_B | ghn_A | ghn_B]
    All three gi matmuls are N=512 and prefetchable ahead of the scan.
  - Two 256-wide batch chains hide the serial per-step latency.
"""

import numpy as np

import concourse.bass as bass
import concourse.tile as tile
import concourse.mybir as mybir
from concourse.bass_utils import run_bass_kernel_spmd

N_CORES = 8
B, T, V, E, H, FC, O = 4096, 64, 512, 32, 128, 256, 1024
BS = B // N_CORES      # batch per core
TS = T - 1             # message slices consumed by the GRU
NCH = 2                # batch chains per core
CW = BS // NCH         # chain width
LOOK = 6               # emb pipeline lookahead (steps)
EMB_D = 8              # emb_aug ring depth

F16 = mybir.dt.float16
F32 = mybir.dt.float32
AF = mybir.ActivationFunctionType
OP = mybir.AluOpType

_PROGRAM = None
LAST_RESULTS = None


# walrus codegen encodes at most 1 sem wait per instruction; excess waits
# are hoisted onto NoOp carriers on the same engine (engines execute their
# queues in order, so waiting earlier on the same engine is equivalent).
_WAIT_LIMITS: dict = {}
_DEFAULT_WAIT_LIMIT = 1


def _split_excess_waits(nc):
    for f in nc.m.functions:
        for bb in f.blocks:
            newlist = []
            changed = False
            for inst in bb.instructions:
                si = inst.sync_info
                limit = _WAIT_LIMITS.get(type(inst).__name__, _DEFAULT_WAIT_LIMIT)
                if (
                    limit is not None
                    and si is not None
                    and si.on_wait is not None
                    and len(si.on_wait) > limit
                ):
                    waits = list(si.on_wait)
                    for k, w in enumerate(waits[:-limit]):
                        carrier = mybir.InstNoOp(
                            name=f"{inst.name}-wsplit{k}", ins=[], outs=[]
                        )
                        carrier.engine = inst.engine
                        carrier.sync_info = mybir.SyncInfo(on_wait=[w], on_update=[])
                        newlist.append(carrier)
                    si.on_wait = waits[-limit:]
                    inst.sync_info = si
                    changed = True
                newlist.append(inst)
            if changed:
                bb.instructions[:] = newlist


def _build_program():
    nc = bass.Bass()

    msg = nc.dram_tensor("msg", [TS, 128, V // 128, BS], F16, kind="ExternalInput")
    wembT = nc.dram_tensor("wembT", [V, E], F16, kind="ExternalInput")
    wihT = nc.dram_tensor("wihT", [E + 1, 3 * H], F16, kind="ExternalInput")
    whhT = nc.dram_tensor("whhT", [H, 3 * H], F16, kind="ExternalInput")
    wfc1T = nc.dram_tensor("wfc1T", [H, FC], F16, kind="ExternalInput")
    wfc2T = nc.dram_tensor("wfc2T", [FC, O], F16, kind="ExternalInput")
    # bias columns: 0 mh1, 1 rh-stt, 2..3 fc1, 4..11 fc2
    biasd = nc.dram_tensor("bias", [128, 12], F32, kind="ExternalInput")
    out = nc.dram_tensor("out", [O // 128, 128, BS], F32, kind="ExternalOutput")

    with tile.TileContext(nc) as tc:
        with (
            tc.tile_pool(name="const", bufs=1) as const,
            tc.tile_pool(name="msgp", bufs=8) as msgp,
            tc.tile_pool(name="rzs", bufs=3) as rzsp,
            tc.tile_pool(name="gate", bufs=3) as gate,
            tc.tile_pool(name="fcp", bufs=2) as fcp,
            tc.tile_pool(name="outp", bufs=2) as outp,
            tc.tile_pool(name="ps_big1", bufs=2, space="PSUM") as ps_big1,
            tc.tile_pool(name="ps_big2", bufs=1, space="PSUM") as ps_big2,
            tc.tile_pool(name="ps_emb", bufs=2, space="PSUM") as ps_emb,
        ):
            # ---- resident constants ----
            wemb_sb = const.tile([128, V // 128, E], F16)
            nc.sync.dma_start(wemb_sb[:], wembT.rearrange("(c p) e -> p c e", p=128))
            wih_sb = const.tile([E + 1, 3 * H], F16)
            nc.sync.dma_start(wih_sb[:], wihT[:])
            whh_sb = const.tile([H, 3 * H], F16)
            nc.sync.dma_start(whh_sb[:], whhT[:])
            wfc1_sb = const.tile([H, FC], F16)
            nc.sync.dma_start(wfc1_sb[:], wfc1T[:])
            wfc2_sb = const.tile([128, FC // 128, O], F16)
            nc.sync.dma_start(wfc2_sb[:], wfc2T.rearrange("(c p) o -> p c o", p=128))
            bias_sb = const.tile([128, 12], F32)
            nc.sync.dma_start(bias_sb[:], biasd[:])
            zeros = const.tile([128, BS], F16)
            nc.gpsimd.memset(zeros[:], 0.0)

            # emb_aug ring: rows 0..31 emb (rewritten per slot), row 32 a
            # persistent 1.0 (bias contraction row), written once here.
            emb_ring = []
            for i in range(EMB_D):
                tl = const.tile([E + 1, BS], F16, name=f"embr{i}", tag=f"embr{i}")
                nc.gpsimd.memset(tl[E : E + 1, :], 1.0)
                emb_ring.append(tl)

            # mh after step 0 is batch-independent (host-computed) -> broadcast.
            mh_tiles = [[None, None] for _ in range(NCH)]
            for c in range(NCH):
                for j in range(2):
                    t0 = const.tile([H, CW], F16, name=f"mh{c}_{j}",
                                    tag=f"mh{c}_{j}")
                    mh_tiles[c][j] = t0
                nc.vector.tensor_scalar_add(mh_tiles[c][0][:], zeros[:, 0:CW],
                                            bias_sb[:, 0:1])

            def emb_front(k):
                """DMA message slice k and project it to emb_ring[k % EMB_D]."""
                mt = msgp.tile([128, V // 128, BS], F16)
                nc.sync.dma_start(mt[:], msg[k])
                ep = ps_emb.tile([E, BS], F32)
                for c4 in range(V // 128):
                    nc.tensor.matmul(
                        ep[:],
                        wemb_sb[:, c4, :],
                        mt[:, c4, :],
                        start=(c4 == 0),
                        stop=(c4 == V // 128 - 1),
                    )
                es = emb_ring[k % EMB_D]
                nc.scalar.copy(es[0:E, :], ep[:])

            for k in range(T + LOOK):
                if k < TS:
                    emb_front(k)

                st = k - LOOK
                if not (1 <= st <= T - 1):
                    continue
                es = emb_ring[(st - 1) % EMB_D]
                big1 = ps_big1.tile([128, 4 * CW], F32, tag="big1")
                big2 = ps_big2.tile([128, 4 * CW], F32, tag="big2")
                b1v = big1.rearrange("p (two half) -> p two half", two=2)
                # --- input-side matmuls: wide (both chains), prefetchable.
                # One accumulation chain per PSUM bank: gi_r opens bank1a,
                # gi_z opens bank1b, gh_* only accumulate (start=False).
                nc.tensor.matmul(big2[:, 0:2 * CW], wih_sb[:, 2 * H:3 * H],
                                 es[:], start=True, stop=True)
                nc.tensor.matmul(big1[:, 0:2 * CW], wih_sb[:, 0:H], es[:],
                                 start=True, stop=False)
                nc.tensor.matmul(big1[:, 2 * CW:4 * CW], wih_sb[:, H:2 * H],
                                 es[:], start=True, stop=False)
                for c in range(NCH):
                    sl = bass.ts(c, CW)
                    mh_prev = mh_tiles[c][(st - 1) % 2]
                    mh_new = mh_tiles[c][st % 2]

                    nc.tensor.matmul(big1[:, c * CW:(c + 1) * CW],
                                     whh_sb[:, 0:H], mh_prev[:],
                                     start=False, stop=True)
                    nc.tensor.matmul(big1[:, (2 + c) * CW:(3 + c) * CW],
                                     whh_sb[:, H:2 * H], mh_prev[:],
                                     start=False, stop=True)
                    nc.tensor.matmul(big2[:, (2 + c) * CW:(3 + c) * CW],
                                     whh_sb[:, 2 * H:3 * H], mh_prev[:],
                                     start=True, stop=True)

                    # [r | zc] in one sigmoid (z-weights pre-negated);
                    # strided src pairs this chain's r and zc banks.
                    rzs = rzsp.tile([128, 2 * CW], F16, tag="rzs" + str(c))
                    nc.scalar.activation(
                        rzs.rearrange("p (two half) -> p two half", two=2),
                        b1v[:, :, c * CW:(c + 1) * CW], AF.Sigmoid)
                    # rh = (2*ghn' + c_hn) .* r
                    rh = gate.tile([128, CW], F16, tag="rh" + str(c))
                    nc.vector.scalar_tensor_tensor(
                        rh[:], big2[:, (2 + c) * CW:(3 + c) * CW],
                        bias_sb[:, 1:2], rzs[:, 0:CW],
                        op0=OP.add, op1=OP.mult)
                    # s2 = 2*gin' + rh  (= 2s)
                    s2 = gate.tile([128, CW], F16, tag="s2" + str(c))
                    nc.vector.tensor_tensor(s2[:], big2[:, c * CW:(c + 1) * CW],
                                            rh[:], op=OP.add)
                    # n^ = sigmoid(2s)
                    nh = gate.tile([128, CW], F16, tag="nh" + str(c))
                    nc.scalar.activation(nh[:], s2[:], AF.Sigmoid)
                    # d = n^ - mh ; e = zc .* d ; mh' = mh + e
                    d = gate.tile([128, CW], F16, tag="d" + str(c))
                    nc.vector.tensor_tensor(d[:], nh[:], mh_prev[:],
                                            op=OP.subtract)
                    e = gate.tile([128, CW], F16, tag="e" + str(c))
                    nc.vector.tensor_tensor(e[:], rzs[:, CW:2 * CW], d[:],
                                            op=OP.mult)
                    nc.vector.tensor_tensor(mh_new[:], mh_prev[:], e[:],
                                            op=OP.add)

            # ---- output head: fc1 + ELU, fc2 + sigmoid ----
            mh_last = [mh_tiles[c][(T - 1) % 2] for c in range(NCH)]
            hid = []
            for f in range(FC // 128):
                pf = ps_big1.tile([128, 4 * CW], F32, tag="big1")
                for c in range(NCH):
                    nc.tensor.matmul(pf[:, bass.ts(c, CW)],
                                     wfc1_sb[:, bass.ts(f, 128)],
                                     mh_last[c][:], start=True, stop=True)
                bcol = bias_sb[:, 2 + f:3 + f]
                x1 = fcp.tile([128, BS], F16, tag="fcx")
                nc.vector.tensor_scalar_add(x1[:], pf[:, 0:BS], bcol)
                e1 = fcp.tile([128, BS], F32, tag="fce")
                nc.scalar.activation(e1[:], pf[:, 0:BS], AF.Exp, bias=bcol)
                # elu(x) = max(x,0) + min(exp(x)-1, 0)
                em = fcp.tile([128, BS], F16, tag="fcm")
                nc.vector.scalar_tensor_tensor(em[:], e1[:], -1.0, zeros[:],
                                               op0=OP.add, op1=OP.min)
                hc = fcp.tile([128, BS], F16, tag="hid" + str(f))
                nc.vector.scalar_tensor_tensor(hc[:], x1[:], 0.0, em[:],
                                               op0=OP.max, op1=OP.add)
                hid.append(hc)
            for o in range(O // 128):
                po = ps_big1.tile([128, 4 * CW], F32, tag="big1")
                for f in range(FC // 128):
                    nc.tensor.matmul(po[:, 0:BS], wfc2_sb[:, f, bass.ts(o, 128)],
                                     hid[f][:], start=(f == 0),
                                     stop=(f == FC // 128 - 1))
                ob = outp.tile([128, BS], F32)
                nc.scalar.activation(ob[:], po[:, 0:BS], AF.Sigmoid,
                                     bias=bias_sb[:, 4 + o:5 + o])
                nc.sync.dma_start(out[o], ob[:])

    _split_excess_waits(nc)
    return nc


def _sigmoid(x):
    return 1.0 / (1.0 + np.exp(-x))


def kernel(message, W_emb, b_emb, init_emb, W_ih, W_hh, b_ih, b_hh,
           W_fc1, b_fc1, W_fc2, b_fc2, _trace=False, _trace_kwargs=None):
    global _PROGRAM, LAST_RESULTS
    if _PROGRAM is None:
        _PROGRAM = _build_program()
    nc = _PROGRAM

    f32 = np.float32
    f16 = np.float16
    f64 = np.float64

    # message -> per-core [t, p, c4, b] fp16 (v = c4*128 + p); the final
    # token's embedding is never consumed by the GRU.
    m16 = message.astype(f16)
    msgT = np.ascontiguousarray(
        m16[:, :TS, :]
        .reshape(N_CORES, BS, TS, V // 128, 128)
        .transpose(0, 2, 4, 3, 1)
    )

    Wih_r, Wih_z, Wih_n = (W_ih[:H].astype(f64), W_ih[H:2 * H].astype(f64),
                           W_ih[2 * H:].astype(f64))
    Whh_r, Whh_z, Whh_n = (W_hh[:H].astype(f64), W_hh[H:2 * H].astype(f64),
                           W_hh[2 * H:].astype(f64))
    b_emb64 = b_emb.astype(f64)
    b_r = b_ih[:H].astype(f64) + b_hh[:H] + Wih_r @ b_emb64
    b_z = b_ih[H:2 * H].astype(f64) + b_hh[H:2 * H] + Wih_z @ b_emb64
    b_in = b_ih[2 * H:].astype(f64) + Wih_n @ b_emb64
    b_hn = b_hh[2 * H:].astype(f64)

    # mh-state folds (h = 2*mh - 1): double W_hh, subtract its row sums
    bias_row = np.concatenate([
        b_r - Whh_r.sum(1),
        -(b_z - Whh_z.sum(1)),
        2.0 * b_in,
    ])
    wih_eff = np.concatenate([Wih_r, -Wih_z, 2.0 * Wih_n], 0)       # [3H, E]
    whh_dev = 2.0 * np.concatenate([Whh_r, -Whh_z, 2.0 * Whh_n], 0)  # [3H, H]
    wihT = np.ascontiguousarray(
        np.concatenate([wih_eff, bias_row[:, None]], 1).T).astype(f16)
    whhT = np.ascontiguousarray(whh_dev.T).astype(f16)
    wembT = np.ascontiguousarray(W_emb.T).astype(f16)
    wfc1T = np.ascontiguousarray((2.0 * W_fc1.astype(f64)).T).astype(f16)
    wfc2T = np.ascontiguousarray(W_fc2.T).astype(f16)

    # step 0 is batch-independent: h0 = 0, x0 = init_emb
    gi0 = W_ih.astype(f64) @ init_emb[0, 0].astype(f64) + b_ih
    r0 = _sigmoid(gi0[0:H] + b_hh[0:H])
    z0 = _sigmoid(gi0[H:2 * H] + b_hh[H:2 * H])
    n0 = np.tanh(gi0[2 * H:3 * H] + r0 * b_hh[2 * H:3 * H])
    h1 = (1.0 - z0) * n0

    bias = np.zeros((128, 12), f32)
    bias[:, 0] = (1.0 + h1) / 2.0
    bias[:, 1] = 2.0 * (b_hn - Whh_n.sum(1))
    bias[:, 2:4] = (b_fc1.astype(f64) - W_fc1.astype(f64).sum(1)) \
        .reshape(2, 128).T.astype(f32)
    bias[:, 4:12] = b_fc2.reshape(8, 128).T.astype(f32)

    shared = dict(wembT=wembT, wihT=wihT, whhT=whhT, wfc1T=wfc1T,
                  wfc2T=wfc2T, bias=bias)
    in_maps = [dict(msg=msgT[c], **shared) for c in range(N_CORES)]

    kw = dict(_trace_kwargs or {})
    res = run_bass_kernel_spmd(nc, in_maps, list(range(N_CORES)),
                               trace=_trace, **kw)
    LAST_RESULTS = res

    outs = [res.results[c]["out"].reshape(O, BS).T for c in range(N_CORES)]
    return np.ascontiguousarray(np.concatenate(outs, axis=0), dtype=f32)
